# revision 42
# baseline (speedup 1.0000x reference)
"""Trainium2 Bass kernel for causal multi-head attention (B=2, S=2048, E=1024, H=16).

Sharding: 8 cores = 2 batches x 4 head-groups (4 heads each).
Each core computes its batch's QKV for its 4 heads, causal attention, and a
partial output projection; host sums the 4 group partials per batch, then adds
b_out and the (softmax-invariant-factored) W_out @ b_v term.

Key design points (vs the fp32r baseline, 289us -> ~195us):
- All matmul operands bf16 (same PE rate as fp32r at >=256 moving rows, but
  full rate at ANY size -> finer causal trimming; half DVE/DMA traffic).
  PSUM accumulation stays fp32; y partials leave as bf16.
- Score matmuls have K=64, so the two heads of a pair auto-place into
  distinct PE row-groups (tile_position) and run concurrently.
- k-bias dropped (softmax-invariant), v-bias folded into a host-side
  W_out @ b_v add, q-bias and 1/sqrt(hd) folded into W_q/b_q on the host.
- Causal mask applied as a bf16 0/1 multiply on the exp output (SBUF-only,
  2x DVE mode) instead of a -1e30 add on fp32 PSUM.
- Softmax denominators ride as a 65th "ones" column of the V operand; their
  reciprocal runs wide on the DVE after a [1,2N] -> [128,N/64] DMA reshape
  (sub-128-partition ops are lane-starved otherwise), then is DMA-broadcast
  and multiplied into pairt in place (DVE/gpsimd, SBUF-only).
- Rollout is split per head-pair: hp0's normalize chain resolves during
  hp1's attention loop, and the shared 2-bank po accumulator rotates between
  pairs.  The last chunk runs a per-query-tile rollout + output projection +
  y store pipelined against the final PV flush to kill the tail.
- All inputs are prefetched up front (per-ke interleaved so chunk-0 QKV
  starts ~1.5us in); next-chunk QKV and prev-chunk out-proj matmuls are
  woven between attention iterations to keep the PE HAM-warm (2.4 GHz).
"""
import sys

sys.path.insert(0, "/opt/trn_rl_repo")

from contextlib import ExitStack

import ml_dtypes
import numpy as np

import concourse.bass as bass
import concourse.tile as tile
from concourse import bacc, mybir
from concourse.bass_utils import run_bass_kernel_spmd

dt = mybir.dt
AF = mybir.ActivationFunctionType

B, S, E, H = 2, 2048, 1024, 16
HD = 64                     # head dim
HPC = 4                     # heads per core
NC = 8                      # cores
KE = E // 128               # 8 contraction k-tiles for projections
NT = S // 128               # 16 token tiles
NCH = S // 512              # 4 token chunks
PEND = 4                    # pv emission delay (iterations)



DEBUG_OUTS = False


def _build_program():
    nc = bacc.Bacc("TRN2", target_bir_lowering=False, debug=False, num_devices=NC)

    xT_d = nc.dram_tensor("xT", [E, S], dt.bfloat16, kind="ExternalInput")
    wqkT_d = nc.dram_tensor("wqkT", [E, 512], dt.bfloat16, kind="ExternalInput")
    wvT_d = nc.dram_tensor("wvT", [E, 256], dt.bfloat16, kind="ExternalInput")
    bq_d = nc.dram_tensor("bq", [256], dt.float32, kind="ExternalInput")
    wo_d = nc.dram_tensor("wo", [256, E], dt.bfloat16, kind="ExternalInput")
    mask_d = nc.dram_tensor("trimask", [128, 128], dt.float32, kind="ExternalInput")
    y_d = nc.dram_tensor("y", [S, E], dt.bfloat16, kind="ExternalOutput")

    dbg = {}
    if DEBUG_OUTS:
        dbg["qkt"] = nc.dram_tensor("dbg_qkt", [4, 128, S], dt.bfloat16, kind="ExternalOutput")
        dbg["pair"] = nc.dram_tensor("dbg_pair", [2, 128, S], dt.bfloat16, kind="ExternalOutput")
        dbg["den"] = nc.dram_tensor("dbg_den", [NCH, 2048], dt.float32, kind="ExternalOutput")
        dbg["recip"] = nc.dram_tensor("dbg_recip", [NCH, 2048], dt.bfloat16, kind="ExternalOutput")
        dbg["bcs"] = nc.dram_tensor("dbg_bcs", [NCH, 128, 1024], dt.bfloat16, kind="ExternalOutput")
        dbg["vones"] = nc.dram_tensor("dbg_vones", [NT, 128, 260], dt.bfloat16, kind="ExternalOutput")

    with TileKernel(nc) as tk:
        tk.dbg = dbg
        tk.build(xT_d, wqkT_d, wvT_d, bq_d, wo_d, mask_d, y_d)
    nc.compile()
    return nc


class TileKernel:
    def __init__(self, nc):
        self.nc = nc
        self.dbg = {}
        self.ctx = ExitStack()
        self.tc_cm = tile.TileContext(nc)

    def __enter__(self):
        self.tc = self.tc_cm.__enter__()
        return self

    def __exit__(self, *a):
        self.ctx.close()
        return self.tc_cm.__exit__(*a)

    def build(self, xT_d, wqkT_d, wvT_d, bq_d, wo_d, mask_d, y_d):
        nc, tc, ctx = self.nc, self.tc, self.ctx
        pool = lambda name, bufs, **kw: ctx.enter_context(
            tc.tile_pool(name=name, bufs=bufs, **kw)
        )

        const_p = pool("const", 1)
        xs_p = pool("xs", 1)
        qkt_p = pool("qkt", 1)
        vones_p = pool("vones", 1)
        attn_p = pool("attn", PEND + 2)
        pair_p = pool("pair", 1)
        small_p = pool("small", 2)
        y_p = pool("y", 3)
        ps_p = pool("ps", 3, space="PSUM")     # 3 x [128,1024] = 6 banks
        po_p = pool("po", 1, space="PSUM")     # [65, 2, 512] shared = 2 banks

        # ---- small consts first (fast DMAs) ----
        mask_sb = const_p.tile([128, 128], dt.bfloat16, tag="mask")
        nc.gpsimd.dma_start(mask_sb[:], mask_d[:])
        bq_sb = const_p.tile([128, 2], dt.float32, tag="bq")
        nc.sync.dma_start(bq_sb[:], bq_d[:].rearrange("(f p) -> p f", p=128))

        # ---- weights + x, interleaved per-ke so chunk-0 QKV starts early ----
        wqk_sb = const_p.tile([128, KE, 512], dt.bfloat16, tag="wqk")
        xs = xs_p.tile([128, KE, S], dt.bfloat16, tag="xs", name="xs")
        for ke in range(KE):
            nc.scalar.dma_start(
                wqk_sb[:, ke, :],
                wqkT_d[128 * ke : 128 * (ke + 1), :],
            )
            nc.sync.dma_start(
                xs[:, ke, 0:512],
                xT_d[128 * ke : 128 * (ke + 1), 0:512],
            )
        wv_sb = const_p.tile([128, KE, 256], dt.bfloat16, tag="wv")
        nc.sync.dma_start(
            wv_sb[:],
            wvT_d[:].rearrange("(ke p) f -> p ke f", p=128),
        )
        for c in range(1, NCH):
            if c == 2:
                wo_sb = const_p.tile([128, 2, E], dt.bfloat16, tag="wo")
                nc.sync.dma_start(
                    wo_sb[:],
                    wo_d[:].rearrange("(kt p) f -> p kt f", p=128),
                )
            cs = slice(512 * c, 512 * (c + 1))
            nc.sync.dma_start(
                xs[:, :, cs],
                xT_d[:, cs].rearrange("(ke p) f -> p ke f", p=128),
            )

        # ---- persistent activations ----
        # qkt tiles: 0: q heads 0,1 | 1: q heads 2,3 | 2: k heads 0,1 | 3: k heads 2,3
        qkt = [qkt_p.tile([128, S], dt.bfloat16, tag=f"qkt{f}", name=f"qkt{f}")
               for f in range(4)]
        # vones[t]: per head [v(64) | 1] -> [128, 4, 65]
        vones = [vones_p.tile([128, 4, 65], dt.bfloat16, tag=f"v{t}", name=f"v{t}")
                 for t in range(NT)]
        for t in range(NT):
            nc.vector.memset(vones[t][:, :, 64:65], 1.0)
        # pairt[kt]: normalized attn output, [2 heads x 64 dims, S]
        pairt = [pair_p.tile([128, S], dt.bfloat16, tag=f"pair{hp}", name=f"pair{hp}")
                 for hp in range(2)]

        env = dict(
            xs=xs, wqk_sb=wqk_sb, wv_sb=wv_sb, bq_sb=bq_sb, wo_sb=wo_sb,
            mask_sb=mask_sb, qkt=qkt, vones=vones, pairt=pairt,
            xs_p=xs_p, ps_p=ps_p, po_p=po_p, attn_p=attn_p, small_p=small_p,
            y_p=y_p, y_d=y_d, po={}, recip={},
        )

        # startup: chunk-0 qkv emitted directly
        for u in self.qkv_units(0, env):
            u()
        for c in range(NCH):
            fillers = []
            if c == 1:
                fillers += self.r2_units(0, env) + self.qkv_units(2, env)
            elif c == 2:
                fillers += self.r2_units(1, env) + self.oproj_units(0, env)
                fillers += self.qkv_units(3, env)
            elif c == 3:
                fillers += self.r2_units(2, env)
                fillers += self.oproj_units(1, env) + self.oproj_units(2, env)
            elif c == 0:
                fillers += self.qkv_units(1, env)
            self.attention_chunk(c, env, fillers)
        if self.dbg:
            for f in range(4):
                nc.sync.dma_start(self.dbg["qkt"][f], qkt[f][:])
            for hp in range(2):
                nc.sync.dma_start(self.dbg["pair"][hp], pairt[hp][:])
            for t in range(NT):
                nc.sync.dma_start(
                    self.dbg["vones"][t],
                    vones[t][:].rearrange("p g d -> p (g d)"),
                )

    # ------------------------------------------------------------------
    def qkv_units(self, c, env):
        nc = self.nc
        cs = slice(512 * c, 512 * (c + 1))
        xs, wqk_sb, wv_sb = env["xs"], env["wqk_sb"], env["wv_sb"]
        bq_sb, qkt, vones = env["bq_sb"], env["qkt"], env["vones"]
        ps_p = env["ps_p"]
        units = []

        def qk_unit(f):
            pq = ps_p.tile([128, 1024], dt.float32, tag="ps", name="pq")
            for ke in range(KE):
                nc.tensor.matmul(
                    pq[:, 0:512],
                    wqk_sb[:, ke, 128 * f : 128 * (f + 1)],
                    xs[:, ke, cs],
                    start=(ke == 0), stop=(ke == KE - 1),
                )
            if f < 2:
                nc.vector.tensor_scalar_add(
                    qkt[f][:, cs], pq[:, 0:512], bq_sb[:, f : f + 1])
            else:
                nc.vector.tensor_copy(qkt[f][:, cs], pq[:, 0:512])

        def v_unit(t4):
            t = 4 * c + t4
            pv = ps_p.tile([128, 1024], dt.float32, tag="ps", name="pv")
            for ke in range(KE):
                nc.tensor.matmul(
                    pv[:, 0:256],
                    xs[:, ke, 512 * c + 128 * t4 : 512 * c + 128 * (t4 + 1)],
                    wv_sb[:, ke, :],
                    start=(ke == 0), stop=(ke == KE - 1),
                )
            nc.vector.tensor_copy(
                vones[t][:, :, 0:64],
                pv[:, 0:256].rearrange("p (g d) -> p g d", d=64),
            )

        for f in range(4):
            units.append(lambda f=f: qk_unit(f))
        for t4 in range(4):
            units.append(lambda t4=t4: v_unit(t4))
        return units

    # ------------------------------------------------------------------
    def r2_units(self, c, env):
        """Broadcast reciprocal denominators and normalize pairt in place."""
        nc = self.nc
        pairt = env["pairt"]
        cs = slice(512 * c, 512 * (c + 1))
        bcs = {}

        def bc_unit(hp):
            recrow = env["recip"][(c, hp)]
            sb = env["small_p"].tile([128, 512], dt.bfloat16, tag=f"bcs{hp}",
                                     name="bcs")
            for h in range(2):
                nc.sync.dma_start(
                    sb[64 * h : 64 * (h + 1), :],
                    recrow[0:1, 512 * h : 512 * (h + 1)]
                    .rearrange("a (o n) -> a o n", o=1)
                    .to_broadcast((1, 64, 512)),
                )
            bcs[hp] = sb

        def mult_unit(hp):
            bc = bcs[hp]
            for h in range(2):
                sl = pairt[hp][64 * h : 64 * (h + 1), cs]
                nc.vector.tensor_mul(sl, sl, bc[64 * h : 64 * (h + 1), :])

        return [lambda: bc_unit(0), lambda: mult_unit(0),
                lambda: bc_unit(1), lambda: mult_unit(1)]

    # ------------------------------------------------------------------
    def oproj_units(self, c, env):
        nc = self.nc
        pairt, wo_sb, ps_p, y_p, y_d = (
            env["pairt"], env["wo_sb"], env["ps_p"], env["y_p"], env["y_d"])
        units = []

        def unit(t4):
            t = 4 * c + t4
            ysb = y_p.tile([128, E], dt.bfloat16, tag="y", name="ysb")
            py = ps_p.tile([128, 1024], dt.float32, tag="ps", name="py")
            for o in range(2):
                for kt in range(2):
                    nc.tensor.matmul(
                        py[:, 512 * o : 512 * (o + 1)],
                        pairt[kt][:, 128 * t : 128 * (t + 1)],
                        wo_sb[:, kt, 512 * o : 512 * (o + 1)],
                        start=(kt == 0), stop=(kt == 1),
                    )
            nc.vector.tensor_copy(ysb[:], py[:])
            eng = nc.sync if t % 2 == 0 else nc.scalar
            eng.dma_start(y_d[128 * t : 128 * (t + 1), :], ysb[:])

        for t4 in range(4):
            units.append(lambda t4=t4: unit(t4))
        return units

    # ------------------------------------------------------------------
    def attention_chunk(self, c, env, fillers):
        """Attention for both head pairs of chunk c, weaving filler units
        (prev-chunk rollout/out-proj, next-chunk qkv) into the PE stream."""
        nc = self.nc
        qkt, vones, mask_sb = env["qkt"], env["vones"], env["mask_sb"]
        ps_p, po_p, attn_p = env["ps_p"], env["po_p"], env["attn_p"]
        nj = 4 * c + 4

        nfill = len(fillers)
        iters = 2 * nj
        emitted = 0

        def emit_pv(hp, j, off, at):
            for h in range(2):
                i = 2 * hp + h
                nc.tensor.matmul(
                    po_t[hp][:, h, off:512],
                    vones[j][:, i, :],
                    at[:, 512 * h + off : 512 * (h + 1)],
                    start=(j == 0), stop=(j == nj - 1),
                    skip_group_check=True,
                )

        it = 0
        po_t = [None, None]
        for hp in range(2):
            po_t[hp] = po_p.tile([65, 2, 512], dt.float32, tag="po", name="po")
            pending = []
            for j in range(nj):
                at = attn_p.tile([128, 1024], dt.bfloat16, tag="attn", name="at")
                ps = ps_p.tile([128, 1024], dt.float32, tag="ps", name="ps")
                m = j - 4 * c
                off = 128 * m if m >= 1 else 0
                for h in range(2):
                    r0 = 64 * h
                    nc.tensor.matmul(
                        ps[:, 512 * h + off : 512 * (h + 1)],
                        qkt[2 + hp][r0 : r0 + 64, 128 * j : 128 * (j + 1)],
                        qkt[hp][r0 : r0 + 64, 512 * c + off : 512 * (c + 1)],
                        start=True, stop=True,
                    )
                if off == 0:
                    nc.scalar.activation(at[:], ps[:], AF.Exp)
                else:
                    for h in range(2):
                        nc.scalar.activation(
                            at[:, 512 * h + off : 512 * (h + 1)],
                            ps[:, 512 * h + off : 512 * (h + 1)], AF.Exp)
                if m >= 0:
                    for h in range(2):
                        lo = 512 * h + 128 * m
                        nc.vector.tensor_mul(
                            at[:, lo : lo + 128], at[:, lo : lo + 128],
                            mask_sb[:],
                        )
                pending.append((j, off, at))
                if len(pending) > PEND:
                    emit_pv(hp, *pending.pop(0))
                it += 1
                while emitted < nfill and emitted * iters < it * nfill:
                    fillers[emitted]()
                    emitted += 1
            if c == NCH - 1 and hp == 1:
                for p in pending:
                    emit_pv(hp, *p)
                    self.rollout_qtile(c, p[0] - 4 * c, po_t[1], env)
            else:
                for p in pending:
                    emit_pv(hp, *p)
                self.rollout_hp(c, hp, po_t[hp], env)
                if c == NCH - 1:
                    # hp0 normalize for the last chunk, resolved during hp1
                    r2 = self.r2_units(c, env)
                    r2[0](); r2[1]()
        while emitted < nfill:
            fillers[emitted]()
            emitted += 1

    def rollout_hp(self, c, hp, po, env):
        """Copy unnormalized attn output to pairt (freeing po) and compute
        1/denominator: ACT Copy (no table switch) -> DMA reshape to [128,8]
        -> wide DVE reciprocal -> DMA back to a row for broadcasting."""
        nc = self.nc
        pairt = env["pairt"]
        cs = slice(512 * c, 512 * (c + 1))
        for h in range(2):
            nc.vector.tensor_copy(
                pairt[hp][64 * h : 64 * (h + 1), cs], po[0:64, h, :])
        denrow = env["small_p"].tile([1, 1024], dt.float32, tag=f"recf{hp}",
                                     name="denrow")
        nc.scalar.activation(denrow[0:1, :], po[64:65, :, :], AF.Copy)
        dencol = env["small_p"].tile([128, 8], dt.float32, tag=f"denc{hp}",
                                     name="dencol")
        nc.sync.dma_start(dencol[:], denrow[0:1, :])
        reccol = env["small_p"].tile([128, 8], dt.float32, tag=f"recc{hp}",
                                     name="reccol")
        nc.vector.reciprocal(reccol[:], dencol[:])
        reccol_bf = env["small_p"].tile([128, 8], dt.bfloat16, tag=f"reccb{hp}",
                                        name="reccol_bf")
        nc.vector.tensor_copy(reccol_bf[:], reccol[:])
        recrow = env["small_p"].tile([1, 1024], dt.bfloat16, tag=f"recb{hp}",
                                     name="recrow")
        nc.sync.dma_start(recrow[0:1, :], reccol_bf[:])
        env["recip"][(c, hp)] = recrow

    def rollout_qtile(self, c, m, po, env):
        """Last-chunk hp1: normalize one query tile and immediately run its
        output projection + y store, pipelined against the remaining PVs."""
        nc = self.nc
        pairt, ps_p, y_p, y_d = env["pairt"], env["ps_p"], env["y_p"], env["y_d"]
        wo_sb = env["wo_sb"]
        t = 4 * c + m
        qs = slice(128 * t, 128 * (t + 1))
        ms = slice(128 * m, 128 * (m + 1))
        for h in range(2):
            nc.vector.tensor_copy(
                pairt[1][64 * h : 64 * (h + 1), qs], po[0:64, h, ms])
        sp = env["small_p"]
        q = nc.sync if m % 2 == 0 else nc.scalar
        denrow = sp.tile([1, 256], dt.float32, tag=f"qden{m}", name="denrow")
        nc.scalar.activation(denrow[0:1, :], po[64:65, :, ms], AF.Copy)
        dencol = sp.tile([32, 8], dt.float32, tag=f"qdenc{m}", name="dencol")
        q.dma_start(dencol[:], denrow[0:1, :])
        reccol = sp.tile([32, 8], dt.float32, tag=f"qrecc{m}", name="reccol")
        nc.vector.reciprocal(reccol[:], dencol[:])
        reccol_bf = sp.tile([32, 8], dt.bfloat16, tag=f"qreccb{m}", name="reccol_bf")
        nc.vector.tensor_copy(reccol_bf[:], reccol[:])
        recrow = sp.tile([1, 256], dt.bfloat16, tag=f"qrecb{m}", name="recrow")
        q.dma_start(recrow[0:1, :], reccol_bf[:])
        bcsq = sp.tile([128, 128], dt.bfloat16, tag=f"qbcs{m}", name="bcsq")
        for h in range(2):
            q.dma_start(
                bcsq[64 * h : 64 * (h + 1), :],
                recrow[0:1, 128 * h : 128 * (h + 1)]
                .rearrange("a (o n) -> a o n", o=1)
                .to_broadcast((1, 64, 128)),
            )
        for h in range(2):
            sl = pairt[1][64 * h : 64 * (h + 1), qs]
            nc.vector.tensor_mul(sl, sl, bcsq[64 * h : 64 * (h + 1), :])
        ysb = y_p.tile([128, E], dt.bfloat16, tag="y", name="ysb")
        py = ps_p.tile([128, 1024], dt.float32, tag="ps", name="py")
        for o in range(2):
            for kt in range(2):
                nc.tensor.matmul(
                    py[:, 512 * o : 512 * (o + 1)],
                    pairt[kt][:, qs],
                    wo_sb[:, kt, 512 * o : 512 * (o + 1)],
                    start=(kt == 0), stop=(kt == 1),
                )
        nc.vector.tensor_copy(ysb[:], py[:])
        nc.scalar.dma_start(y_d[qs, :], ysb[:])
        if self.dbg:
            den_sb = env["small_p"].tile([1, 2048], dt.float32, tag="dens", name="dens")
            nc.vector.tensor_copy(den_sb[0:1, :], po[64:65, :, :])
            nc.sync.dma_start(self.dbg["den"][c], den_sb[0:1, :].rearrange("a n -> (a n)"))
            nc.sync.dma_start(self.dbg["recip"][c], recip_bf[0:1, :].rearrange("a n -> (a n)"))


# ----------------------------------------------------------------------
_PROGRAM = None


def _get_program():
    global _PROGRAM
    if _PROGRAM is None:
        _PROGRAM = _build_program()
    return _PROGRAM


def _make_in_maps(inputs, W_in, b_in, W_out, b_out):
    in_maps = []
    bf16 = ml_dtypes.bfloat16
    scale = 1.0 / np.sqrt(np.float32(HD))
    kr = np.arange(128)[:, None]
    qc = np.arange(128)[None, :]
    trimask = np.where(qc >= kr, 1.0, 0.0).astype(np.float32)
    for core in range(NC):
        b, g = divmod(core, 4)
        r = slice(256 * g, 256 * (g + 1))
        wq = W_in[0:E][r] * scale
        wk = W_in[E : 2 * E][r]
        wv = W_in[2 * E : 3 * E][r]
        xT = np.ascontiguousarray(inputs[b].T).astype(bf16)
        wqkT = np.ascontiguousarray(np.concatenate([wq, wk], axis=0).T).astype(bf16)
        wvT = np.ascontiguousarray(wv.T).astype(bf16)
        bq = (b_in[0:E][r] * scale).astype(np.float32)
        wo = np.ascontiguousarray(W_out[:, r].T).astype(bf16)
        in_maps.append(
            {
                "xT": xT,
                "wqkT": wqkT,
                "wvT": wvT,
                "bq": bq,
                "wo": wo,
                "trimask": trimask,
            }
        )
    return in_maps


def run_spmd(inputs, W_in, b_in, W_out, b_out, trace=False, **kw):
    nc = _get_program()
    in_maps = _make_in_maps(inputs, W_in, b_in, W_out, b_out)
    bkr = run_bass_kernel_spmd(nc, in_maps, list(range(NC)), trace=trace, **kw)
    parts = [bkr.results[i]["y"].astype(np.float32) for i in range(NC)]
    out = np.stack(
        [
            parts[0] + parts[1] + parts[2] + parts[3],
            parts[4] + parts[5] + parts[6] + parts[7],
        ]
    )
    yb = W_out.astype(np.float32) @ b_in[2 * E : 3 * E].astype(np.float32)
    out = out + (yb + b_out)[None, None, :]
    return out.astype(np.float32), bkr


def kernel(inputs, W_in, b_in, W_out, b_out):
    out, _ = run_spmd(
        np.asarray(inputs, dtype=np.float32),
        np.asarray(W_in, dtype=np.float32),
        np.asarray(b_in, dtype=np.float32),
        np.asarray(W_out, dtype=np.float32),
        np.asarray(b_out, dtype=np.float32),
    )
    return out


# revision 43
# speedup vs baseline: 1.1300x; 1.1300x over previous
"""Trainium2 Bass kernel for causal multi-head attention (B=2, S=2048, E=1024, H=16).

Sharding: 8 cores = 2 batches x 4 head-groups (4 heads each).
Each core computes its batch's QKV for its 4 heads, causal attention, and a
partial output projection; host sums the 4 group partials per batch, then adds
b_out and the (softmax-invariant-factored) W_out @ b_v term.

Key design points (vs the fp32r baseline, 289us -> ~195us):
- All matmul operands bf16 (same PE rate as fp32r at >=256 moving rows, but
  full rate at ANY size -> finer causal trimming; half DVE/DMA traffic).
  PSUM accumulation stays fp32; y partials leave as bf16.
- Score matmuls have K=64, so the two heads of a pair auto-place into
  distinct PE row-groups (tile_position) and run concurrently.
- k-bias dropped (softmax-invariant), v-bias folded into a host-side
  W_out @ b_v add, q-bias and 1/sqrt(hd) folded into W_q/b_q on the host.
- Causal mask applied as a bf16 0/1 multiply on the exp output (SBUF-only,
  2x DVE mode) instead of a -1e30 add on fp32 PSUM.
- Softmax denominators ride as a 65th "ones" column of the V operand; their
  reciprocal runs wide on the DVE after a [1,2N] -> [128,N/64] DMA reshape
  (sub-128-partition ops are lane-starved otherwise), then is DMA-broadcast
  and multiplied into pairt in place (DVE/gpsimd, SBUF-only).
- Rollout is split per head-pair: hp0's normalize chain resolves during
  hp1's attention loop, and the shared 2-bank po accumulator rotates between
  pairs.  The last chunk runs a per-query-tile rollout + output projection +
  y store pipelined against the final PV flush to kill the tail.
- All inputs are prefetched up front (per-ke interleaved so chunk-0 QKV
  starts ~1.5us in); next-chunk QKV and prev-chunk out-proj matmuls are
  woven between attention iterations to keep the PE HAM-warm (2.4 GHz).
"""
import sys

sys.path.insert(0, "/opt/trn_rl_repo")

from contextlib import ExitStack

import ml_dtypes
import numpy as np

import concourse.bass as bass
import concourse.tile as tile
from concourse import bacc, mybir
from concourse.bass_utils import run_bass_kernel_spmd

dt = mybir.dt
AF = mybir.ActivationFunctionType

B, S, E, H = 2, 2048, 1024, 16
HD = 64                     # head dim
HPC = 4                     # heads per core
NC = 8                      # cores
KE = E // 128               # 8 contraction k-tiles for projections
NT = S // 128               # 16 token tiles
NCH = S // 512              # 4 token chunks
PEND = 4                    # pv emission delay (iterations)



DEBUG_OUTS = False


def _build_program():
    nc = bacc.Bacc("TRN2", target_bir_lowering=False, debug=False, num_devices=NC)

    xT_d = nc.dram_tensor("xT", [E, S], dt.bfloat16, kind="ExternalInput")
    wqkT_d = nc.dram_tensor("wqkT", [E, 512], dt.bfloat16, kind="ExternalInput")
    wvT_d = nc.dram_tensor("wvT", [E, 256], dt.bfloat16, kind="ExternalInput")
    bq_d = nc.dram_tensor("bq", [256], dt.float32, kind="ExternalInput")
    wo_d = nc.dram_tensor("wo", [256, E], dt.bfloat16, kind="ExternalInput")
    mask_d = nc.dram_tensor("trimask", [128, 128], dt.float32, kind="ExternalInput")
    y_d = nc.dram_tensor("y", [S, E], dt.bfloat16, kind="ExternalOutput")

    dbg = {}
    if DEBUG_OUTS:
        dbg["qkt"] = nc.dram_tensor("dbg_qkt", [4, 128, S], dt.bfloat16, kind="ExternalOutput")
        dbg["pair"] = nc.dram_tensor("dbg_pair", [2, 128, S], dt.bfloat16, kind="ExternalOutput")
        dbg["den"] = nc.dram_tensor("dbg_den", [NCH, 2048], dt.float32, kind="ExternalOutput")
        dbg["recip"] = nc.dram_tensor("dbg_recip", [NCH, 2048], dt.bfloat16, kind="ExternalOutput")
        dbg["bcs"] = nc.dram_tensor("dbg_bcs", [NCH, 128, 1024], dt.bfloat16, kind="ExternalOutput")
        dbg["vones"] = nc.dram_tensor("dbg_vones", [NT, 128, 260], dt.bfloat16, kind="ExternalOutput")

    with TileKernel(nc) as tk:
        tk.dbg = dbg
        tk.build(xT_d, wqkT_d, wvT_d, bq_d, wo_d, mask_d, y_d)
    nc.compile()
    return nc


class TileKernel:
    def __init__(self, nc):
        self.nc = nc
        self.dbg = {}
        self.ctx = ExitStack()
        self.tc_cm = tile.TileContext(nc)

    def __enter__(self):
        self.tc = self.tc_cm.__enter__()
        return self

    def __exit__(self, *a):
        self.ctx.close()
        return self.tc_cm.__exit__(*a)

    def build(self, xT_d, wqkT_d, wvT_d, bq_d, wo_d, mask_d, y_d):
        nc, tc, ctx = self.nc, self.tc, self.ctx
        pool = lambda name, bufs, **kw: ctx.enter_context(
            tc.tile_pool(name=name, bufs=bufs, **kw)
        )

        const_p = pool("const", 1)
        xs_p = pool("xs", 1)
        qkt_p = pool("qkt", 1)
        vones_p = pool("vones", 1)
        attn_p = pool("attn", PEND + 2)
        pair_p = pool("pair", 1)
        small_p = pool("small", 2)
        y_p = pool("y", 3)
        ps_p = pool("ps", 3, space="PSUM")     # 3 x [128,1024] = 6 banks
        po_p = pool("po", 1, space="PSUM")     # [65, 2, 512] shared = 2 banks

        # ---- small consts first (fast DMAs) ----
        mask_sb = const_p.tile([128, 128], dt.bfloat16, tag="mask")
        nc.gpsimd.dma_start(mask_sb[:], mask_d[:])
        bq_sb = const_p.tile([128, 2], dt.float32, tag="bq")
        nc.sync.dma_start(bq_sb[:], bq_d[:].rearrange("(f p) -> p f", p=128))

        # ---- weights + x, interleaved per-ke so chunk-0 QKV starts early ----
        wqk_sb = const_p.tile([128, KE, 512], dt.bfloat16, tag="wqk")
        xs = xs_p.tile([128, KE, S], dt.bfloat16, tag="xs", name="xs")
        for ke in range(KE):
            nc.sync.dma_start(
                wqk_sb[:, ke, :],
                wqkT_d[128 * ke : 128 * (ke + 1), :],
            )
            nc.sync.dma_start(
                xs[:, ke, 0:512],
                xT_d[128 * ke : 128 * (ke + 1), 0:512],
            )
        wv_sb = const_p.tile([128, KE, 256], dt.bfloat16, tag="wv")
        nc.sync.dma_start(
            wv_sb[:],
            wvT_d[:].rearrange("(ke p) f -> p ke f", p=128),
        )
        for c in range(1, NCH):
            if c == 2:
                wo_sb = const_p.tile([128, 2, E], dt.bfloat16, tag="wo")
                nc.sync.dma_start(
                    wo_sb[:],
                    wo_d[:].rearrange("(kt p) f -> p kt f", p=128),
                )
            cs = slice(512 * c, 512 * (c + 1))
            nc.sync.dma_start(
                xs[:, :, cs],
                xT_d[:, cs].rearrange("(ke p) f -> p ke f", p=128),
            )

        # ---- persistent activations ----
        # qkt tiles: 0: q heads 0,1 | 1: q heads 2,3 | 2: k heads 0,1 | 3: k heads 2,3
        qkt = [qkt_p.tile([128, S], dt.bfloat16, tag=f"qkt{f}", name=f"qkt{f}")
               for f in range(4)]
        # vones[t]: per head [v(64) | 1] -> [128, 4, 65]
        vones = [vones_p.tile([128, 4, 65], dt.bfloat16, tag=f"v{t}", name=f"v{t}")
                 for t in range(NT)]
        for t in range(NT):
            nc.vector.memset(vones[t][:, :, 64:65], 1.0)
        # pairt[kt]: normalized attn output, [2 heads x 64 dims, S]
        pairt = [pair_p.tile([128, S], dt.bfloat16, tag=f"pair{hp}", name=f"pair{hp}")
                 for hp in range(2)]

        env = dict(
            xs=xs, wqk_sb=wqk_sb, wv_sb=wv_sb, bq_sb=bq_sb, wo_sb=wo_sb,
            mask_sb=mask_sb, qkt=qkt, vones=vones, pairt=pairt,
            xs_p=xs_p, ps_p=ps_p, po_p=po_p, attn_p=attn_p, small_p=small_p,
            y_p=y_p, y_d=y_d, po={}, recip={},
        )

        # startup: chunk-0 qkv emitted directly
        for u in self.qkv_units(0, env):
            u()
        for c in range(NCH):
            fillers = []
            if c == 1:
                fillers += self.r2_units(0, env) + self.qkv_units(2, env)
            elif c == 2:
                fillers += self.r2_units(1, env) + self.oproj_units(0, env)
                fillers += self.qkv_units(3, env)
            elif c == 3:
                fillers += self.r2_units(2, env)
                fillers += self.oproj_units(1, env) + self.oproj_units(2, env)
            elif c == 0:
                fillers += self.qkv_units(1, env)
            self.attention_chunk(c, env, fillers)
        if self.dbg:
            for f in range(4):
                nc.sync.dma_start(self.dbg["qkt"][f], qkt[f][:])
            for hp in range(2):
                nc.sync.dma_start(self.dbg["pair"][hp], pairt[hp][:])
            for t in range(NT):
                nc.sync.dma_start(
                    self.dbg["vones"][t],
                    vones[t][:].rearrange("p g d -> p (g d)"),
                )

    # ------------------------------------------------------------------
    def qkv_units(self, c, env):
        nc = self.nc
        cs = slice(512 * c, 512 * (c + 1))
        xs, wqk_sb, wv_sb = env["xs"], env["wqk_sb"], env["wv_sb"]
        bq_sb, qkt, vones = env["bq_sb"], env["qkt"], env["vones"]
        ps_p = env["ps_p"]
        units = []

        def qk_unit(f):
            pq = ps_p.tile([128, 1024], dt.float32, tag="ps", name="pq")
            for ke in range(KE):
                nc.tensor.matmul(
                    pq[:, 0:512],
                    wqk_sb[:, ke, 128 * f : 128 * (f + 1)],
                    xs[:, ke, cs],
                    start=(ke == 0), stop=(ke == KE - 1),
                )
            if f < 2:
                nc.vector.tensor_scalar_add(
                    qkt[f][:, cs], pq[:, 0:512], bq_sb[:, f : f + 1])
            else:
                nc.vector.tensor_copy(qkt[f][:, cs], pq[:, 0:512])

        def v_unit(t4):
            t = 4 * c + t4
            pv = ps_p.tile([128, 1024], dt.float32, tag="ps", name="pv")
            for ke in range(KE):
                nc.tensor.matmul(
                    pv[:, 0:256],
                    xs[:, ke, 512 * c + 128 * t4 : 512 * c + 128 * (t4 + 1)],
                    wv_sb[:, ke, :],
                    start=(ke == 0), stop=(ke == KE - 1),
                )
            nc.vector.tensor_copy(
                vones[t][:, :, 0:64],
                pv[:, 0:256].rearrange("p (g d) -> p g d", d=64),
            )

        for f in range(4):
            units.append(lambda f=f: qk_unit(f))
        for t4 in range(4):
            units.append(lambda t4=t4: v_unit(t4))
        return units

    # ------------------------------------------------------------------
    def r2_units(self, c, env):
        """Broadcast reciprocal denominators and normalize pairt in place."""
        nc = self.nc
        pairt = env["pairt"]
        cs = slice(512 * c, 512 * (c + 1))
        bcs = {}

        def bc_unit(hp):
            recrow = env["recip"][(c, hp)]
            sb = env["small_p"].tile([128, 512], dt.bfloat16, tag=f"bcs{hp}",
                                     name="bcs")
            for h in range(2):
                nc.sync.dma_start(
                    sb[64 * h : 64 * (h + 1), :],
                    recrow[0:1, 512 * h : 512 * (h + 1)]
                    .rearrange("a (o n) -> a o n", o=1)
                    .to_broadcast((1, 64, 512)),
                )
            bcs[hp] = sb

        def mult_unit(hp):
            bc = bcs[hp]
            for h in range(2):
                sl = pairt[hp][64 * h : 64 * (h + 1), cs]
                nc.vector.tensor_mul(sl, sl, bc[64 * h : 64 * (h + 1), :])

        return [lambda: bc_unit(0), lambda: mult_unit(0),
                lambda: bc_unit(1), lambda: mult_unit(1)]

    # ------------------------------------------------------------------
    def oproj_units(self, c, env):
        nc = self.nc
        pairt, wo_sb, ps_p, y_p, y_d = (
            env["pairt"], env["wo_sb"], env["ps_p"], env["y_p"], env["y_d"])
        units = []

        def unit(t4):
            t = 4 * c + t4
            ysb = y_p.tile([128, E], dt.bfloat16, tag="y", name="ysb")
            py = ps_p.tile([128, 1024], dt.float32, tag="ps", name="py")
            for o in range(2):
                for kt in range(2):
                    nc.tensor.matmul(
                        py[:, 512 * o : 512 * (o + 1)],
                        pairt[kt][:, 128 * t : 128 * (t + 1)],
                        wo_sb[:, kt, 512 * o : 512 * (o + 1)],
                        start=(kt == 0), stop=(kt == 1),
                    )
            nc.vector.tensor_copy(ysb[:], py[:])
            nc.gpsimd.dma_start(y_d[128 * t : 128 * (t + 1), :], ysb[:])

        for t4 in range(4):
            units.append(lambda t4=t4: unit(t4))
        return units

    # ------------------------------------------------------------------
    def attention_chunk(self, c, env, fillers):
        """Attention for both head pairs of chunk c, weaving filler units
        (prev-chunk rollout/out-proj, next-chunk qkv) into the PE stream."""
        nc = self.nc
        qkt, vones, mask_sb = env["qkt"], env["vones"], env["mask_sb"]
        ps_p, po_p, attn_p = env["ps_p"], env["po_p"], env["attn_p"]
        nj = 4 * c + 4

        nfill = len(fillers)
        iters = 2 * nj
        emitted = 0

        def emit_pv(hp, j, off, at):
            for h in range(2):
                i = 2 * hp + h
                nc.tensor.matmul(
                    po_t[hp][:, h, off:512],
                    vones[j][:, i, :],
                    at[:, 512 * h + off : 512 * (h + 1)],
                    start=(j == 0), stop=(j == nj - 1),
                    skip_group_check=True,
                )

        it = 0
        po_t = [None, None]
        for hp in range(2):
            po_t[hp] = po_p.tile([65, 2, 512], dt.float32, tag="po", name="po")
            pending = []
            for j in range(nj):
                at = attn_p.tile([128, 1024], dt.bfloat16, tag="attn", name="at")
                ps = ps_p.tile([128, 1024], dt.float32, tag="ps", name="ps")
                m = j - 4 * c
                off = 128 * m if m >= 1 else 0
                for h in range(2):
                    r0 = 64 * h
                    nc.tensor.matmul(
                        ps[:, 512 * h + off : 512 * (h + 1)],
                        qkt[2 + hp][r0 : r0 + 64, 128 * j : 128 * (j + 1)],
                        qkt[hp][r0 : r0 + 64, 512 * c + off : 512 * (c + 1)],
                        start=True, stop=True,
                    )
                if off == 0:
                    nc.scalar.activation(at[:], ps[:], AF.Exp)
                else:
                    for h in range(2):
                        nc.scalar.activation(
                            at[:, 512 * h + off : 512 * (h + 1)],
                            ps[:, 512 * h + off : 512 * (h + 1)], AF.Exp)
                if m >= 0:
                    for h in range(2):
                        lo = 512 * h + 128 * m
                        nc.vector.tensor_mul(
                            at[:, lo : lo + 128], at[:, lo : lo + 128],
                            mask_sb[:],
                        )
                pending.append((j, off, at))
                if len(pending) > PEND:
                    emit_pv(hp, *pending.pop(0))
                it += 1
                while emitted < nfill and emitted * iters < it * nfill:
                    fillers[emitted]()
                    emitted += 1
            if c == NCH - 1 and hp == 1:
                for p in pending:
                    emit_pv(hp, *p)
                    self.rollout_qtile(c, p[0] - 4 * c, po_t[1], env)
            else:
                for p in pending:
                    emit_pv(hp, *p)
                self.rollout_hp(c, hp, po_t[hp], env)
                if c == NCH - 1:
                    # hp0 normalize for the last chunk, resolved during hp1
                    r2 = self.r2_units(c, env)
                    r2[0](); r2[1]()
        while emitted < nfill:
            fillers[emitted]()
            emitted += 1

    def rollout_hp(self, c, hp, po, env):
        """Copy unnormalized attn output to pairt (freeing po) and compute
        1/denominator: ACT Copy (no table switch) -> DMA reshape to [128,8]
        -> wide DVE reciprocal -> DMA back to a row for broadcasting."""
        nc = self.nc
        pairt = env["pairt"]
        cs = slice(512 * c, 512 * (c + 1))
        for h in range(2):
            nc.vector.tensor_copy(
                pairt[hp][64 * h : 64 * (h + 1), cs], po[0:64, h, :])
        denrow = env["small_p"].tile([1, 1024], dt.float32, tag=f"recf{hp}",
                                     name="denrow")
        nc.scalar.activation(denrow[0:1, :], po[64:65, :, :], AF.Copy)
        dencol = env["small_p"].tile([128, 8], dt.float32, tag=f"denc{hp}",
                                     name="dencol")
        nc.sync.dma_start(dencol[:], denrow[0:1, :])
        reccol = env["small_p"].tile([128, 8], dt.float32, tag=f"recc{hp}",
                                     name="reccol")
        nc.vector.reciprocal(reccol[:], dencol[:])
        reccol_bf = env["small_p"].tile([128, 8], dt.bfloat16, tag=f"reccb{hp}",
                                        name="reccol_bf")
        nc.vector.tensor_copy(reccol_bf[:], reccol[:])
        recrow = env["small_p"].tile([1, 1024], dt.bfloat16, tag=f"recb{hp}",
                                     name="recrow")
        nc.sync.dma_start(recrow[0:1, :], reccol_bf[:])
        env["recip"][(c, hp)] = recrow

    def rollout_qtile(self, c, m, po, env):
        """Last-chunk hp1: normalize one query tile and immediately run its
        output projection + y store, pipelined against the remaining PVs."""
        nc = self.nc
        pairt, ps_p, y_p, y_d = env["pairt"], env["ps_p"], env["y_p"], env["y_d"]
        wo_sb = env["wo_sb"]
        t = 4 * c + m
        qs = slice(128 * t, 128 * (t + 1))
        ms = slice(128 * m, 128 * (m + 1))
        for h in range(2):
            nc.vector.tensor_copy(
                pairt[1][64 * h : 64 * (h + 1), qs], po[0:64, h, ms])
        sp = env["small_p"]
        q = nc.sync if m % 2 == 0 else nc.scalar
        denrow = sp.tile([1, 256], dt.float32, tag=f"qden{m}", name="denrow")
        nc.scalar.activation(denrow[0:1, :], po[64:65, :, ms], AF.Copy)
        dencol = sp.tile([32, 8], dt.float32, tag=f"qdenc{m}", name="dencol")
        q.dma_start(dencol[:], denrow[0:1, :])
        reccol = sp.tile([32, 8], dt.float32, tag=f"qrecc{m}", name="reccol")
        nc.vector.reciprocal(reccol[:], dencol[:])
        reccol_bf = sp.tile([32, 8], dt.bfloat16, tag=f"qreccb{m}", name="reccol_bf")
        nc.vector.tensor_copy(reccol_bf[:], reccol[:])
        recrow = sp.tile([1, 256], dt.bfloat16, tag=f"qrecb{m}", name="recrow")
        q.dma_start(recrow[0:1, :], reccol_bf[:])
        bcsq = sp.tile([128, 128], dt.bfloat16, tag=f"qbcs{m}", name="bcsq")
        for h in range(2):
            q.dma_start(
                bcsq[64 * h : 64 * (h + 1), :],
                recrow[0:1, 128 * h : 128 * (h + 1)]
                .rearrange("a (o n) -> a o n", o=1)
                .to_broadcast((1, 64, 128)),
            )
        for h in range(2):
            sl = pairt[1][64 * h : 64 * (h + 1), qs]
            nc.vector.tensor_mul(sl, sl, bcsq[64 * h : 64 * (h + 1), :])
        ysb = y_p.tile([128, E], dt.bfloat16, tag="y", name="ysb")
        py = ps_p.tile([128, 1024], dt.float32, tag="ps", name="py")
        for o in range(2):
            for kt in range(2):
                nc.tensor.matmul(
                    py[:, 512 * o : 512 * (o + 1)],
                    pairt[kt][:, qs],
                    wo_sb[:, kt, 512 * o : 512 * (o + 1)],
                    start=(kt == 0), stop=(kt == 1),
                )
        nc.vector.tensor_copy(ysb[:], py[:])
        nc.scalar.dma_start(y_d[qs, :], ysb[:])
        if self.dbg:
            den_sb = env["small_p"].tile([1, 2048], dt.float32, tag="dens", name="dens")
            nc.vector.tensor_copy(den_sb[0:1, :], po[64:65, :, :])
            nc.sync.dma_start(self.dbg["den"][c], den_sb[0:1, :].rearrange("a n -> (a n)"))
            nc.sync.dma_start(self.dbg["recip"][c], recip_bf[0:1, :].rearrange("a n -> (a n)"))


# ----------------------------------------------------------------------
_PROGRAM = None


def _get_program():
    global _PROGRAM
    if _PROGRAM is None:
        _PROGRAM = _build_program()
    return _PROGRAM


def _make_in_maps(inputs, W_in, b_in, W_out, b_out):
    in_maps = []
    bf16 = ml_dtypes.bfloat16
    scale = 1.0 / np.sqrt(np.float32(HD))
    kr = np.arange(128)[:, None]
    qc = np.arange(128)[None, :]
    trimask = np.where(qc >= kr, 1.0, 0.0).astype(np.float32)
    for core in range(NC):
        b, g = divmod(core, 4)
        r = slice(256 * g, 256 * (g + 1))
        wq = W_in[0:E][r] * scale
        wk = W_in[E : 2 * E][r]
        wv = W_in[2 * E : 3 * E][r]
        xT = np.ascontiguousarray(inputs[b].T).astype(bf16)
        wqkT = np.ascontiguousarray(np.concatenate([wq, wk], axis=0).T).astype(bf16)
        wvT = np.ascontiguousarray(wv.T).astype(bf16)
        bq = (b_in[0:E][r] * scale).astype(np.float32)
        wo = np.ascontiguousarray(W_out[:, r].T).astype(bf16)
        in_maps.append(
            {
                "xT": xT,
                "wqkT": wqkT,
                "wvT": wvT,
                "bq": bq,
                "wo": wo,
                "trimask": trimask,
            }
        )
    return in_maps


def run_spmd(inputs, W_in, b_in, W_out, b_out, trace=False, **kw):
    nc = _get_program()
    in_maps = _make_in_maps(inputs, W_in, b_in, W_out, b_out)
    bkr = run_bass_kernel_spmd(nc, in_maps, list(range(NC)), trace=trace, **kw)
    parts = [bkr.results[i]["y"].astype(np.float32) for i in range(NC)]
    out = np.stack(
        [
            parts[0] + parts[1] + parts[2] + parts[3],
            parts[4] + parts[5] + parts[6] + parts[7],
        ]
    )
    yb = W_out.astype(np.float32) @ b_in[2 * E : 3 * E].astype(np.float32)
    out = out + (yb + b_out)[None, None, :]
    return out.astype(np.float32), bkr


def kernel(inputs, W_in, b_in, W_out, b_out):
    out, _ = run_spmd(
        np.asarray(inputs, dtype=np.float32),
        np.asarray(W_in, dtype=np.float32),
        np.asarray(b_in, dtype=np.float32),
        np.asarray(W_out, dtype=np.float32),
        np.asarray(b_out, dtype=np.float32),
    )
    return out
